# revision 17
# baseline (speedup 1.0000x reference)
"""Causal attention with ALiBi, tensor-parallel over (batch, head-group) on 8 NeuronCores.

Sharding: core c handles batch c//4 and head group g=c%4 = heads [12+g, 8+g, 4+g, g]
(one head per "slot"; slot j has a uniform ALiBi band width so all cores run the
same SPMD program). Megatron-style: each core computes a partial output
projection over its 4 heads; partials are summed on the host.

Softmax trick: scores are computed directly in exp-ready form by augmenting the
Q/K feature dim with 2 extra rows so that the matmul yields
    qk/8 + slope*(j - i) - C   (j=key idx, i=query idx, C=4)
which is the true score minus the analytic row max (slope*(i-2047) cancels in
softmax). No row-max pass, no row-sum pass: a ones-column appended to V makes
the AV matmul emit the softmax denominator as PSUM row 64.
"""
import ml_dtypes
import numpy as np

import concourse.bacc as bacc
import concourse.mybir as mybir
import concourse.tile as tile
from concourse.bass_utils import run_bass_kernel_spmd

B, T, D, H, HD = 2, 2048, 1024, 16, 64
NCORES = 8
C_SUB = 4.0          # analytic max margin
MASK_NEG = -1e9
P = 128
TB = 512             # q-block width
NTB = T // TB        # 4
NTT = T // P         # 16
NDC = D // P         # 8
# slot -> number of k-tiles below (and including) the diagonal, from
# gamma=20 band cutoff maximized over the heads sharing the slot.
NB = [16, 11, 4, 2]
F32 = mybir.dt.float32
F32R = mybir.dt.float32r
BF16 = mybir.dt.bfloat16

_slopes = [2.0 ** (-0.5 * (h + 1)) for h in range(H)]


def _heads_of_group(g):
    return [12 + g, 8 + g, 4 + g, g]  # slot order 0..3


def build_program(reps=1, phases=3):
    nc = bacc.Bacc("TRN2", target_bir_lowering=False, debug=False)
    xt = nc.dram_tensor("xt", [D, T], BF16, kind="ExternalInput")
    wq = nc.dram_tensor("wq", [D, 4 * HD], BF16, kind="ExternalInput")
    wk = nc.dram_tensor("wk", [D, 4 * HD], BF16, kind="ExternalInput")
    wv = nc.dram_tensor("wv", [D, 4 * HD], BF16, kind="ExternalInput")
    wo = nc.dram_tensor("wo", [4 * HD, D], F32R, kind="ExternalInput")
    qaug = nc.dram_tensor("qaug", [4, 3, T], F32R, kind="ExternalInput")
    kaug = nc.dram_tensor("kaug", [4, 3, T], F32R, kind="ExternalInput")
    cmadd = nc.dram_tensor("cmadd", [P, 896], F32, kind="ExternalInput")
    cmmul = nc.dram_tensor("cmmul", [P, 896], F32R, kind="ExternalInput")
    y = nc.dram_tensor("y", [T, D], F32, kind="ExternalOutput")

    xt_r = xt[:].rearrange("(dc p) t -> p dc t", p=P)
    wq_r = wq[:].rearrange("(dc p) e -> p dc e", p=P)
    wk_r = wk[:].rearrange("(dc p) e -> p dc e", p=P)
    wv_r = wv[:].rearrange("(dc p) e -> p dc e", p=P)
    wo_r = wo[:].rearrange("(pp p) n -> p pp n", p=P)
    y_r = y[:].rearrange("(tt p) n -> p tt n", p=P)

    with tile.TileContext(nc) as tc:
        with (
            tc.tile_pool(name="big", bufs=1) as big,        # long-lived tensors
            tc.tile_pool(name="stage", bufs=3) as stage,    # qk staging
            tc.tile_pool(name="pt", bufs=6) as ptp,         # exp(scores)
            tc.tile_pool(name="small", bufs=3) as small,    # rc / rcb / stg2
            tc.tile_pool(name="yp", bufs=3) as ypool,       # y sbuf staging
            tc.tile_pool(name="psA", bufs=4, space="PSUM") as psA,   # [128,512]
            tc.tile_pool(name="psV", bufs=1, space="PSUM") as psV,   # [128,256]
            tc.tile_pool(name="psC", bufs=2, space="PSUM") as psC,   # [65,512]
            tc.tile_pool(name="psB", bufs=1, space="PSUM") as psB,   # [64,512]
        ):
            def body(_=None):
                # ---- resident tensors -------------------------------------
                xt_sb = [big.tile([P, T], BF16, tag=f"xt{dc}", name=f"xt{dc}")
                         for dc in range(NDC)]
                for dc in range(NDC):
                    nc.sync.dma_start(xt_sb[dc][:], xt_r[:, dc, :])
                wq_sb = big.tile([P, NDC, 4 * HD], BF16, tag="wq")
                wk_sb = big.tile([P, NDC, 4 * HD], BF16, tag="wk")
                wv_sb = big.tile([P, NDC, 4 * HD], BF16, tag="wv")
                nc.sync.dma_start(wq_sb[:], wq_r)
                nc.sync.dma_start(wk_sb[:], wk_r)
                nc.sync.dma_start(wv_sb[:], wv_r)
                wo_sb = big.tile([P, 2, D], F32R, tag="wo")
                nc.sync.dma_start(wo_sb[:], wo_r)
                cma_sb = big.tile([P, 896], F32, tag="cma")
                cmm_sb = big.tile([P, 896], F32R, tag="cmm")
                nc.sync.dma_start(cma_sb[:], cmadd[:])
                nc.sync.dma_start(cmm_sb[:], cmmul[:])

                qt_sb = [big.tile([67, T], F32R, tag=f"qt{s}", name=f"qt{s}") for s in range(4)]
                kt_sb = [big.tile([67, T], F32R, tag=f"kt{s}", name=f"kt{s}") for s in range(4)]
                for s in range(4):
                    nc.sync.dma_start(qt_sb[s][64:67, :], qaug[s])
                    nc.sync.dma_start(kt_sb[s][64:67, :], kaug[s])
                v_sb = big.tile([P, NTT, 4, HD + 1], F32R, tag="v")
                ones_sb = big.tile([P, 4], F32R, tag="ones")
                nc.sync.dma_start(ones_sb[:], cmmul[:, 892:896])
                ctxT2 = [[big.tile([P, TB], F32R, tag=f"ctx{pp}_{qb}", name=f"ctx{pp}_{qb}")
                          for qb in range(NTB)] for pp in range(2)]

                # ---- phase 1: Q/K/V projections ---------------------------
                for tt in range(NTT):
                    ps = psV.tile([P, 4 * HD], F32, tag="v")
                    for dc in range(NDC):
                        nc.tensor.matmul(
                            ps[:],
                            xt_sb[dc][:, tt * P:(tt + 1) * P],
                            wv_sb[:, dc, :],
                            start=(dc == 0), stop=(dc == NDC - 1),
                        )
                    nc.vector.tensor_copy(
                        v_sb[:, tt, :, 0:HD],
                        ps[:].rearrange("p (s e) -> p s e", s=4))
                    nc.vector.tensor_copy(
                        v_sb[:, tt, :, HD:HD + 1], ones_sb[:, :, None])
                for w_sb, dst, scale in ((wq_sb, qt_sb, 0.125), (wk_sb, kt_sb, 1.0)):
                    for pair in range(2):
                        for tb in range(NTB):
                            ps = psA.tile([P, TB], F32, tag="big")
                            for dc in range(NDC):
                                nc.tensor.matmul(
                                    ps[:],
                                    w_sb[:, dc, pair * P:(pair + 1) * P],
                                    xt_sb[dc][:, tb * TB:(tb + 1) * TB],
                                    start=(dc == 0), stop=(dc == NDC - 1),
                                )
                            tsl = slice(tb * TB, (tb + 1) * TB)
                            # even slot (2*pair): direct copy, partitions 0:64
                            if scale == 1.0:
                                nc.vector.tensor_copy(
                                    dst[2 * pair][0:64, tsl], ps[0:64, :])
                            else:
                                nc.vector.tensor_scalar_mul(
                                    dst[2 * pair][0:64, tsl], ps[0:64, :], scale)
                            # odd slot: stage at partitions 64:128, DMA-shift
                            stg = stage.tile([P, TB], F32R, tag="stg")
                            if scale == 1.0:
                                nc.vector.tensor_copy(
                                    stg[64:128, :], ps[64:128, :])
                            else:
                                nc.vector.tensor_scalar_mul(
                                    stg[64:128, :], ps[64:128, :], scale)
                            nc.sync.dma_start(
                                dst[2 * pair + 1][0:64, tsl], stg[64:128, :])

                # ---- phase 2: banded attention ----------------------------
                for s in (range(4) if phases >= 2 else []):
                    premask = s >= 2  # steep slopes: mask before exp
                    for qb in range(NTB):
                        kt_lo = max(0, 4 * qb - NB[s] + 1)
                        kts = list(range(kt_lo, 4 * qb + 4))
                        pc = psC.tile([65, TB], F32, tag="ctx")
                        qsl = slice(qb * TB, (qb + 1) * TB)
                        for i, kt in enumerate(kts):
                            diag = kt >= 4 * qb
                            off = kt * P - qb * TB if diag else 0
                            w = TB - off  # valid query columns [off, TB)
                            q0 = qb * TB + off
                            ps = psA.tile([P, TB], F32, tag="big")
                            nc.tensor.matmul(
                                ps[:, 0:w],
                                kt_sb[s][:, kt * P:(kt + 1) * P],
                                qt_sb[s][:, q0:qb * TB + TB],
                                start=True, stop=True,
                            )
                            if diag and premask:
                                nc.vector.tensor_tensor(
                                    ps[:, 0:w], ps[:, 0:w],
                                    cma_sb[:, 384:896 - off],
                                    mybir.AluOpType.add)
                            pt = ptp.tile([P, TB], F32R, tag="pt")
                            nc.scalar.activation(
                                pt[:, 0:w], ps[:, 0:w],
                                mybir.ActivationFunctionType.Exp)
                            if diag and not premask:
                                nc.gpsimd.tensor_tensor(
                                    pt[:, 0:w], pt[:, 0:w],
                                    cmm_sb[:, 384:896 - off],
                                    mybir.AluOpType.mult)
                            nc.tensor.matmul(
                                pc[:, off:TB],
                                v_sb[:, kt, s, :],
                                pt[:, 0:w],
                                start=(i == 0), stop=(i == len(kts) - 1),
                                skip_group_check=True,
                            )
                        # epilogue: 1/l, scale ctx, place into ctxT2
                        rc = small.tile([65, TB], F32R, tag="rc")
                        with nc.allow_low_precision(reason="f32r is f32 bits"):
                            nc.vector.reciprocal(rc[64:65, :], pc[64:65, :])
                        # broadcast 1/l across 64 partitions: ones[1,64].T @ rc[1,512]
                        pb = psB.tile([64, TB], F32, tag="rcb")
                        nc.tensor.matmul(
                            pb[:], cmm_sb[64:65, 832:896], rc[64:65, :],
                            start=True, stop=True)
                        rcb = small.tile([64, TB], F32, tag="rcb")
                        nc.scalar.activation(
                            rcb[:], pb[:], mybir.ActivationFunctionType.Copy)
                        pair, odd = divmod(s, 2)
                        if not odd:
                            nc.vector.tensor_tensor(
                                ctxT2[pair][qb][0:64, :], pc[0:64, :], rcb[:],
                                mybir.AluOpType.mult)
                        else:
                            stg2 = small.tile([P, TB], F32R, tag="stg2")
                            nc.vector.tensor_tensor(
                                stg2[0:64, :], pc[0:64, :], rcb[:],
                                mybir.AluOpType.mult)
                            nc.sync.dma_start(
                                ctxT2[pair][qb][64:128, :], stg2[0:64, :])

                # ---- phase 3: output projection ---------------------------
                for tt in (range(NTT) if phases >= 3 else []):
                    qb, tl = divmod(tt, 4)
                    for nh in range(2):
                        py = psA.tile([P, TB], F32, tag="big")
                        nsl = slice(nh * TB, (nh + 1) * TB)
                        for pp in range(2):
                            nc.tensor.matmul(
                                py[:],
                                ctxT2[pp][qb][:, tl * P:(tl + 1) * P],
                                wo_sb[:, pp, nsl],
                                start=(pp == 0), stop=(pp == 1),
                            )
                        ysb = ypool.tile([P, TB], F32, tag="y")
                        if (tt + nh) % 2 == 0:
                            nc.scalar.activation(
                                ysb[:], py[:], mybir.ActivationFunctionType.Copy)
                        else:
                            nc.vector.tensor_copy(ysb[:], py[:])
                        nc.sync.dma_start(y_r[:, tt, nsl], ysb[:])

            if reps == 1:
                body()
            else:
                with tc.For_i(0, reps, 1):
                    body()

    nc.compile()
    return nc


def make_in_maps(x, Wq, bq, Wk, bk, Wv, bv, Wo, bo):
    """Host-side shard prep. Biases are folded: bq/bk/bv must be zero (they are
    in this problem's setup); bo is added after the gather."""
    assert not (np.any(bq) or np.any(bk) or np.any(bv)), "nonzero qkv bias unsupported"
    t_idx = np.arange(T, dtype=np.float64)
    # causal masks for the diagonal 512-blocks: base[p, c] relates key-row p to
    # query-col (c - 384 + off); valid (keep) iff q >= k i.e. c >= p + 384.
    base_keep = (np.arange(896)[None, :] >= (np.arange(P)[:, None] + 384))
    cmadd = np.where(base_keep, 0.0, MASK_NEG).astype(np.float32)
    cmmul = base_keep.astype(np.float32)

    in_maps = []
    for c in range(NCORES):
        b, g = divmod(c, 4)
        heads = _heads_of_group(g)
        cols = np.concatenate([np.arange(h * HD, (h + 1) * HD) for h in heads])
        qa = np.empty((4, 3, T), np.float32)
        ka = np.empty((4, 3, T), np.float32)
        for s, h in enumerate(heads):
            sl = _slopes[h]
            aj = sl * t_idx
            m, e = np.frexp(aj)
            hi = np.ldexp(np.round(m * 256.0) / 256.0, e)  # 8-bit mantissa, f32r-exact
            ka[s, 0] = hi
            ka[s, 1] = (aj - hi).astype(np.float32)
            ka[s, 2] = 1.0
            qa[s, 0] = 1.0
            qa[s, 1] = 1.0
            qa[s, 2] = (-aj - C_SUB).astype(np.float32)
        in_maps.append({
            "xt": np.ascontiguousarray(x[b].T).astype(ml_dtypes.bfloat16),
            "wq": Wq[:, cols].astype(ml_dtypes.bfloat16),
            "wk": Wk[:, cols].astype(ml_dtypes.bfloat16),
            "wv": Wv[:, cols].astype(ml_dtypes.bfloat16),
            "wo": np.ascontiguousarray(Wo[cols, :], dtype=np.float32),
            "qaug": qa, "kaug": ka,
            "cmadd": cmadd, "cmmul": cmmul,
        })
    return in_maps


_prog_cache = {}


def kernel(x, Wq, bq, Wk, bk, Wv, bv, Wo, bo):
    x, Wq, Wk, Wv, Wo = (np.asarray(a, np.float32) for a in (x, Wq, Wk, Wv, Wo))
    bq, bk, bv, bo = (np.asarray(a, np.float32) for a in (bq, bk, bv, bo))
    if "p" not in _prog_cache:
        _prog_cache["p"] = build_program(reps=1)
    nc = _prog_cache["p"]
    in_maps = make_in_maps(x, Wq, bq, Wk, bk, Wv, bv, Wo, bo)
    res = run_bass_kernel_spmd(nc, in_maps, core_ids=list(range(NCORES)))
    out = np.zeros((B, T, D), np.float32)
    for c in range(NCORES):
        out[c // 4] += res.results[c]["y"]
    out += bo
    return out
